# revision 1
# baseline (speedup 1.0000x reference)
"""CometAttention Trainium2 kernel.

Computes, for query [B, D] and values [B, S, D] (B=32, S=2048, D=1024, f32):
    w[b, s]   = (query[b] . values[b, s]) / sqrt(D)
    w         = softmax(w, axis=0)            # over the batch dim!
    out[b,s,:] = values[b,s,:] * w[b,s]

Sharding: S is split across 8 NeuronCores (softmax over B is local to each
(s) column, so an S-shard needs no collectives). Each core gets
values[:, c*256:(c+1)*256, :] plus the full query, and produces the matching
output shard; the host concatenates shards along S.

Per-core layout: s-positions are processed 32 at a time in a [128, 8, 1024]
SBUF tile. Partition block si (32 partitions, one per batch) holds the 8
contiguous s-positions s0+8*si .. s0+8*si+7 on the free dim, with d
innermost (32KB contiguous DMA runs); tile[si*32+b, j, :] = values[b,
s0+8*si+j, :]. The batch-dim softmax denominator is computed with one
TensorE matmul against a block-diagonal ones matrix, which both group-sums
over b and broadcasts the result back to all 32 partitions of each group.

Schedule notes (which the TimelineSim cost model confirms matter):
- loads go through the SP HWDGE ring, stores through the ScalarE ring, so a
  store waiting on the softmax chain can never head-block later loads;
- the weight is applied in place and the vtile doubles as the store source;
- tensor_tensor_reduce is avoided (it faults on this hardware/runtime);
  the dot-product reduction runs as ScalarE Copy-with-accumulate instead.
"""

import os

import numpy as np
from contextlib import ExitStack

# Defensive: recover NeuronCores left wedged by a previous crashed run.
os.environ.setdefault("NEURON_RT_RESET_CORES", "1")

B = 32
S = 2048
D = 1024
N_CORES = 8
S_SH = S // N_CORES        # 256 s-positions per core
SG = 128 // B              # 4 s-positions per 128-partition group
JJ = 8                     # chunks per DMA unit
S_UNIT = SG * JJ           # 16 s-positions per unit
N_UNITS = S_SH // S_UNIT   # 16 units per core

_CACHE: dict = {}


def _build_nc(jj: int = JJ, v_bufs: int = 4, prod_bufs: int = 4,
              wave: int | None = None, taper: tuple = (6, 2),
              chunk_dma: bool = False):
    import concourse.bacc as bacc
    import concourse.mybir as mybir
    import concourse.tile as tile

    f32 = mybir.dt.float32
    Act = mybir.ActivationFunctionType

    nc = bacc.Bacc(
        "TRN2",
        target_bir_lowering=False,
        debug=False,
        enable_asserts=False,
        num_devices=N_CORES,
    )
    values = nc.dram_tensor("values", [B, S_SH, D], f32, kind="ExternalInput")
    query = nc.dram_tensor("query", [B, D], f32, kind="ExternalInput")
    out = nc.dram_tensor("out", [B, S_SH, D], f32, kind="ExternalOutput")
    v_ap, q_ap, o_ap = values.ap(), query.ap(), out.ap()

    with tile.TileContext(nc) as tc, ExitStack() as ctx:
        singles = ctx.enter_context(tc.tile_pool(name="singles", bufs=1))
        vpool = ctx.enter_context(tc.tile_pool(name="vpool", bufs=v_bufs))
        prodpool = ctx.enter_context(tc.tile_pool(name="prodpool", bufs=prod_bufs))
        wpool = ctx.enter_context(tc.tile_pool(name="wpool", bufs=6))
        pspool = ctx.enter_context(tc.tile_pool(name="pspool", bufs=4, space="PSUM"))

        # qtile[si*32 + b, :] = query[b, :]  (loaded via the ScalarE HWDGE
        # ring, which is otherwise idle at startup, so the first values
        # loads on the SP ring begin at t=0)
        qtile = singles.tile([128, D], f32)
        for si in range(SG):
            nc.scalar.dma_start(out=qtile[si * B : (si + 1) * B, :], in_=q_ap)

        # Block-diagonal ones matrix: A[k, m] = 1 iff k//32 == m//32.
        # matmul(out, A, e) then computes out[p, j] = sum_{b in group(p)} e[b, j],
        # i.e. the group sum broadcast back to every partition of the group.
        atile = singles.tile([128, 128], f32)
        nc.vector.memset(atile, 0.0)
        for g in range(SG):
            nc.vector.memset(atile[g * B : (g + 1) * B, g * B : (g + 1) * B], 1.0)

        inv_sqrt_d = 1.0 / float(np.sqrt(D))

        def do_wave(vtile, s0, jj, j_lo, j_hi, osplit=None):
            """Weights + scale + store for chunk range [j_lo, j_hi) of a
            loaded vtile covering s-positions s0 .. s0+SG*jj-1."""
            nw = j_hi - j_lo
            # dot products: wraw[p, j] = sum_d v[p, j, d] * q[b(p), d]/sqrt(D)
            # (DVE elementwise product, then ScalarE copy-with-accumulate for
            # the free-dim reduction; tensor_tensor_reduce faults on this HW)
            wraw = wpool.tile([128, nw], f32, tag="wraw")
            for j in range(j_lo, j_hi):
                prod = prodpool.tile([128, D], f32, tag="prod")
                nc.vector.tensor_mul(prod, vtile[:, j, :], qtile)
                nc.scalar.activation(
                    prod,
                    prod,
                    Act.Copy,
                    scale=inv_sqrt_d,
                    accum_out=wraw[:, j - j_lo : j - j_lo + 1],
                )

            # softmax over b (within each group of 32 partitions)
            etile = wpool.tile([128, nw], f32, tag="etile")
            nc.scalar.activation(etile, wraw, Act.Exp)
            den = pspool.tile([128, nw], f32, tag="den")
            nc.tensor.matmul(den, atile, etile, start=True, stop=True)
            rec = wpool.tile([128, nw], f32, tag="rec")
            nc.vector.reciprocal(rec, den)
            wfin = wpool.tile([128, nw], f32, tag="wfin")
            nc.vector.tensor_mul(wfin, etile, rec)

            # scale values by the per-(b, s) weight, in place (vtile's last
            # reader is the dot-product mul, which already ran), and store.
            # tensor_scalar on DVE runs at 2x for f32 SBUF; ScalarE takes the
            # other half to balance engine occupancy. Stores are issued on the
            # ScalarE HWDGE ring (qActDynamicHW) so their semaphore waits
            # cannot head-block the loads flowing through the SP ring.
            for j in range(j_lo, j_hi):
                if j % 2 == 0:
                    nc.vector.tensor_scalar_mul(
                        vtile[:, j, :], vtile[:, j, :],
                        wfin[:, j - j_lo : j - j_lo + 1]
                    )
                else:
                    nc.scalar.activation(
                        vtile[:, j, :],
                        vtile[:, j, :],
                        Act.Copy,
                        scale=wfin[:, j - j_lo : j - j_lo + 1],
                    )
            if osplit is not None:
                for j in range(j_lo, j_hi):
                    nc.scalar.dma_start(
                        out=osplit[:, :, j, :].transpose([1, 0, 2]),
                        in_=vtile[:, j, :],
                    )
            else:
                for si in range(SG):
                    nc.scalar.dma_start(
                        out=o_ap[:, s0 + jj * si + j_lo : s0 + jj * si + j_hi, :],
                        in_=vtile[si * B : (si + 1) * B, j_lo:j_hi, :],
                    )

        # unit sizes: uniform jj chunks, except optional tapered tail units
        # (smaller final units shorten the post-last-load compute tail)
        sizes = []
        s_total = S_SH // SG  # total chunks per core
        tail = sum(taper)
        assert (s_total - tail) % jj == 0
        sizes = [jj] * ((s_total - tail) // jj) + list(taper)

        w = wave
        s0 = 0
        for ujj in sizes:
            vtile = vpool.tile([128, jj, D], f32, tag="vtile")
            if chunk_dma:
                # one full-width [128, 1024] DMA per chunk: partition block si
                # holds s = s0 + si*ujj + j, so the source AP for chunk j is
                # [si(stride ujj*D), b(stride S_SH*D), d] — 3 dims. Compute on
                # chunk j can start as soon as its own 512KB lands.
                vsplit = v_ap[:, s0 : s0 + SG * ujj, :].rearrange(
                    "b (si j) d -> b si j d", si=SG, j=ujj
                )
                osplit = o_ap[:, s0 : s0 + SG * ujj, :].rearrange(
                    "b (si j) d -> b si j d", si=SG, j=ujj
                )
                for j in range(ujj):
                    nc.sync.dma_start(
                        out=vtile[:, j, :],
                        in_=vsplit[:, :, j, :].transpose([1, 0, 2]),
                    )
            else:
                for si in range(SG):
                    nc.sync.dma_start(
                        out=vtile[si * B : (si + 1) * B, 0:ujj, :],
                        in_=v_ap[:, s0 + ujj * si : s0 + ujj * si + ujj, :],
                    )
            uw = w or ujj
            for j_lo in range(0, ujj, uw):
                do_wave(vtile, s0, ujj, j_lo, min(j_lo + uw, ujj),
                        osplit if chunk_dma else None)
            s0 += SG * ujj

    nc.compile()
    return nc


def _get_nc():
    if "nc" not in _CACHE:
        _CACHE["nc"] = _build_nc()
    return _CACHE["nc"]


def kernel(query: np.ndarray, values: np.ndarray) -> np.ndarray:
    from concourse import bass_utils

    nc = _get_nc()
    query = np.ascontiguousarray(np.asarray(query, dtype=np.float32))
    values = np.asarray(values, dtype=np.float32)
    in_maps = [
        {
            "values": np.ascontiguousarray(values[:, c * S_SH : (c + 1) * S_SH, :]),
            "query": query,
        }
        for c in range(N_CORES)
    ]
    last_exc = None
    for attempt in range(3):
        try:
            res = bass_utils.run_bass_kernel_spmd(
                nc, in_maps, core_ids=list(range(N_CORES))
            )
            return np.concatenate([r["out"] for r in res.results], axis=1)
        except ModuleNotFoundError:
            # BASS_TRACE=1 requests NTFF profiling, whose axon hook module is
            # not shipped in every container; fall back to an untraced run.
            os.environ["BASS_NEVER_TRACE"] = "1"
            last_exc = None
            continue
        except Exception as e:
            # A crashed previous run can leave a NeuronCore transiently
            # wedged (NRT_EXEC_UNIT_UNRECOVERABLE); NEURON_RT_RESET_CORES=1
            # recovers it on a fresh NRT session. Best effort: drop the jax
            # backend so the retry reconnects, and give the previous
            # session's teardown time to finish.
            last_exc = e
            import time as _time

            try:
                import jax.extend as _jex

                _jex.backend.clear_backends()
            except Exception:
                pass
            _time.sleep(20.0)
    raise last_exc

